# revision 17
# baseline (speedup 1.0000x reference)
"""nn_Attn_9715216024104 — sparse attention (MLA + top-k select + sliding window)
on 8 Trainium2 NeuronCores via Bass/Tile.

Sharding: core = b*4 + hg  (b = batch 0..1, hg = head-group 0..3, 4 heads each).
Each core computes its 4 heads' three attention branches end-to-end in bf16
(fp32 PSUM accumulation) and its row-slice of Wproj; the host sums the four
partial [T, C] outputs per batch.

Device layout highlights:
  - activations kept transposed ([dim, T]) so scores come out as S^T[tk, tq];
    softmax row-sums are PE ones-columns appended to the PV lhsT ([V | 1/g]),
    which also folds the branch gate into the normalization.
  - no max-subtraction in softmax (scores are O(1)); masking is additive -600
    pre-scale constants (exp -> exact 0 past bf16).
  - rms norm via Square / ones-matmul / exp(-0.5 ln(.)) (single ACT table set),
    with the norm weights folded into Wcq/Wckv on the host.
  - rope computed as XR*cos -+ XI*sin with real/imag weight columns pre-split
    on the host so the multiplies run at full 128-partition width.

Host side (cheap, O(T*C)): top-k token selection + gather, gate softmax,
transposes, bf16 casts, weight slicing/folding. Device does all O(T^2) work.
"""

import math

import numpy as np
import ml_dtypes

N_HEAD = 16
NOPE, ROPE, VDIM = 32, 64, 32
HD = 96
WINDOW = 128
KEEP = 512
EPS = 1e-6
T, C = 2048, 256
H = 4  # heads per core
NCORES = 8
MASKVAL = -600.0
SCALE = 1.0 / math.sqrt(HD)

BF16 = ml_dtypes.bfloat16


# ---------------------------------------------------------------------------
# host-side helpers
# ---------------------------------------------------------------------------

def _freqs(t=T, dim=ROPE, theta=1e4):
    f = 1.0 / theta ** (np.arange(0, dim, 2, dtype=np.float32) / dim)
    ang = np.outer(np.arange(t, dtype=np.float32), f)
    return np.cos(ang).astype(np.float32), np.sin(ang).astype(np.float32)


def _softmax(s, axis=-1):
    m = np.max(s, axis=axis, keepdims=True)
    e = np.exp(s - m)
    return e / e.sum(axis=axis, keepdims=True)


def _consts():
    """Inline (NEFF-baked) constant tensors."""
    cos, sin = _freqs()
    cos4 = np.tile(cos.T, (H, 1))  # [128, T]
    sin4 = np.tile(sin.T, (H, 1))
    p = np.arange(128)
    mb = np.full((128, 512), MASKVAL, np.float32)
    mb[:, 384:] = np.where(p[:, None] <= p[None, :], 0.0, MASKVAL)
    cc = np.arange(256)
    m3 = np.where((cc[None, :] >= p[:, None]) & (cc[None, :] < p[:, None] + 128),
                  0.0, MASKVAL).astype(np.float32)
    sql = np.zeros((128, 2), np.float32)
    sql[:96, 0] = 1.0   # cq rows
    sql[96:, 1] = 1.0   # ckv rows
    rstdbc = np.zeros((2, 128), np.float32)
    rstdbc[0, :96] = 1.0
    rstdbc[1, 96:] = 1.0
    scl2 = np.array([[1.0 / 96.0, EPS], [1.0 / 32.0, EPS]], np.float32)
    bf = lambda a: a.astype(BF16)
    return dict(cos4=bf(cos4), sin4=bf(sin4), mb=bf(mb), m3=bf(m3),
                sql=bf(sql), rstdbc=bf(rstdbc), scl2=scl2)


def _prep_core_inputs(inputs, b, hg, host):
    """Build the per-core in_map (all bf16; out is f32)."""
    bf = lambda a: np.ascontiguousarray(a).astype(BF16)
    h0 = hg * H
    x = inputs["x"][b]
    m = {}
    m["xT"] = bf(x.T)
    m["selT"] = bf(host["sel"][b].T)
    m["csel"] = bf(np.tile(host["cos"][host["idx"][b]].T, (H, 1)))
    m["ssel"] = bf(np.tile(host["sin"][host["idx"][b]].T, (H, 1)))
    invg = (1.0 / host["gate"][b]).astype(np.float32)  # [3]
    m["invg"] = bf(np.repeat(invg[None, :], 128, 0).repeat(32, 1))  # [128, 96]

    wcq = inputs["Wcq"] * inputs["q_norm_w"][None, :]
    wckv = inputs["Wckv"] * inputs["kv_norm_w"][None, :]
    m["wA"] = bf(np.concatenate([wcq, wckv], 1))  # [256, 128]: cq cols 0-95, ckv 96-127

    wqn3 = inputs["Wq_nope"].reshape(96, N_HEAD, NOPE)[:, h0:h0 + H]
    wqr3 = inputs["Wq_rope"].reshape(96, N_HEAD, ROPE)[:, h0:h0 + H]
    m["wqn"] = bf(wqn3.reshape(96, H * NOPE))
    m["wqxr"] = bf(wqr3[..., :32].reshape(96, H * 32))
    m["wqxi"] = bf(wqr3[..., 32:].reshape(96, H * 32))

    m["wkn"] = bf(inputs["Wk_nope"].reshape(32, N_HEAD, NOPE)[:, h0:h0 + H].reshape(32, H * NOPE))
    m["wv"] = bf(inputs["Wv"].reshape(32, N_HEAD, VDIM)[:, h0:h0 + H].reshape(32, H * VDIM))
    wkr = inputs["Wk_rope"] / N_HEAD
    m["wkrxr"] = bf(wkr[:, :32])
    m["wkrxi"] = bf(wkr[:, 32:])

    for tag, wk_, wv_ in (("s", "Wsel_k", "Wsel_v"), ("w", "Wwin_k", "Wwin_v")):
        k3 = inputs[wk_].reshape(C, N_HEAD, HD)[:, h0:h0 + H]
        m[f"w{tag}n"] = bf(k3[..., :32].reshape(C, H * 32))
        m[f"w{tag}xr"] = bf(k3[..., 32:64].reshape(C, H * 32))
        m[f"w{tag}xi"] = bf(k3[..., 64:].reshape(C, H * 32))
        m[f"w{tag}v"] = bf(inputs[wv_].reshape(C, N_HEAD, VDIM)[:, h0:h0 + H].reshape(C, H * VDIM))

    m["wproj"] = bf(inputs["Wproj"][h0 * VDIM:(h0 + H) * VDIM])  # [128, 256]
    return m


# ---------------------------------------------------------------------------
# device program
# ---------------------------------------------------------------------------

def build_nc(debug_outs=False, fast_recip=True):
    import concourse.bacc as bacc
    import concourse.mybir as mybir
    from concourse.bass import ts, ds
    from concourse.tile import TileContext

    BF = mybir.dt.bfloat16
    F32 = mybir.dt.float32
    AF = mybir.ActivationFunctionType

    nc = bacc.Bacc("TRN2")

    def _recip(out, in_):
        if fast_recip:
            nc.vector.reciprocal_approx_fast(out=out, in_=in_)
        else:
            nc.vector.reciprocal(out=out, in_=in_)

    P = {}
    def inp(name, shape, dt=BF):
        P[name] = nc.declare_dram_parameter(name, list(shape), dt, isOutput=False)

    inp("xT", [C, T]); inp("selT", [C, KEEP])
    inp("csel", [128, KEEP]); inp("ssel", [128, KEEP]); inp("invg", [128, 96])
    inp("wA", [C, 128])
    inp("wqn", [96, 128]); inp("wqxr", [96, 128]); inp("wqxi", [96, 128])
    inp("wkn", [32, 128]); inp("wv", [32, 128])
    inp("wkrxr", [C, 32]); inp("wkrxi", [C, 32])
    inp("wsn", [C, 128]); inp("wsxr", [C, 128]); inp("wsxi", [C, 128]); inp("wsv", [C, 128])
    inp("wwn", [C, 128]); inp("wwxr", [C, 128]); inp("wwxi", [C, 128]); inp("wwv", [C, 128])
    inp("wproj", [128, C])
    out_dram = nc.declare_dram_parameter("out", [T, C], F32, isOutput=True)
    dbg = {}
    if debug_outs:
        for nm, sh in (("d_cqT", [96, T]), ("d_ckvT", [32, T]), ("d_qT0", [96, T]),
                       ("d_k1T0", [96, T]), ("d_kwT0", [96, T]), ("d_ksT0", [96, KEEP]),
                       ("d_vg10", [128, 1024]), ("d_ocT", [128, T])):
            dbg[nm] = nc.declare_dram_parameter(nm, sh, BF, isOutput=True)

    cst = _consts()
    inl = {k: nc.inline_tensor(v, name=f"c_{k}") for k, v in cst.items()}

    NJ = T // 512  # 4 tq chunks of 512

    with TileContext(nc) as tc:
        with (
            tc.tile_pool(name="persist", bufs=1) as pp,
            tc.tile_pool(name="work", bufs=3) as wk,
            tc.tile_pool(name="mtiles", bufs=2) as mt,
        ):
            # ---- load params + consts to SBUF ----
            sb = {}
            for name, dram in P.items():
                sh = list(dram.shape)
                if sh[0] > 128:  # split partition dim
                    tls = []
                    for i in range(sh[0] // 128):
                        t_ = pp.tile([128, sh[1]], BF, name=f"{name}_{i}")
                        nc.sync.dma_start(out=t_[:, :], in_=dram[ts(i, 128), :])
                        tls.append(t_)
                    sb[name] = tls
                else:
                    t_ = pp.tile(sh, BF, name=f"{name}_sb")
                    nc.sync.dma_start(out=t_[:, :], in_=dram[:, :])
                    sb[name] = t_
            for name in ("cos4", "sin4", "mb", "m3", "sql", "rstdbc"):
                d = inl[name]
                t_ = pp.tile(list(d.shape), BF, name=f"{name}_sb")
                nc.sync.dma_start(out=t_[:, :], in_=d[:, :])
                sb[name] = t_
            scl2_sb = pp.tile([2, 2], F32, name="scl2_sb")
            nc.sync.dma_start(out=scl2_sb[:, :], in_=inl["scl2"][:, :])

            xT0, xT1 = sb["xT"]
            selT0, selT1 = sb["selT"]

            # ---- persistent activation tiles ----
            cqT = pp.tile([96, T], BF, name="cqT")
            ckvT = pp.tile([32, T], BF, name="ckvT")
            qT = [pp.tile([96, T], BF, name=f"qT{h}") for h in range(H)]
            k1T = [pp.tile([96, T], BF, name=f"k1T{h}") for h in range(H)]
            kwT = [pp.tile([96, T], BF, name=f"kwT{h}") for h in range(H)]
            ksT = [pp.tile([96, KEEP], BF, name=f"ksT{h}") for h in range(H)]
            vg1 = [pp.tile([128, 16 * 64], BF, name=f"vg1_{h}") for h in range(H)]
            vg2 = [pp.tile([128, 4 * 64], BF, name=f"vg2_{h}") for h in range(H)]
            vg3 = [pp.tile([128, 16 * 64], BF, name=f"vg3_{h}") for h in range(H)]
            ocT = pp.tile([128, T], BF, name="ocT")

            # ================= phase B: cq/ckv + rms =================
            with tc.tile_pool(name="psB", bufs=2, space="PSUM") as psB:
                for jc in range(NJ):
                    cs = ds(512 * jc, 512)
                    pa = psB.tile([128, 512], F32, tag="pa")
                    nc.tensor.matmul(pa[:, :], sb["wA"][0][:, :], xT0[:, cs], start=True, stop=False)
                    nc.tensor.matmul(pa[:, :], sb["wA"][1][:, :], xT1[:, cs], start=False, stop=True)
                    sq = wk.tile([128, 512], BF, tag="sq")
                    nc.scalar.activation(sq[:, :], pa[:, :], AF.Square)
                    pss = psB.tile([2, 512], F32, tag="pss")
                    nc.tensor.matmul(pss[:, :], sb["sql"][:, :], sq[:, :], start=True, stop=True)
                    lnt = wk.tile([2, 512], F32, tag="lnt")
                    nc.scalar.activation(lnt[:, :], pss[:, :], AF.Ln, scale=scl2_sb[0:2, 0:1], bias=scl2_sb[0:2, 1:2])
                    rstd = wk.tile([2, 512], BF, tag="rstd")
                    nc.scalar.activation(rstd[:, :], lnt[:, :], AF.Exp, scale=-0.5)
                    pr = psB.tile([128, 512], F32, tag="pr")
                    nc.tensor.matmul(pr[:, :], sb["rstdbc"][:, :], rstd[:, :], start=True, stop=True)
                    rb = wk.tile([128, 512], BF, tag="rb")
                    nc.scalar.copy(rb[:, :], pr[:, :])
                    nc.vector.tensor_mul(cqT[:, cs], pa[0:96, :], rb[0:96, :])
                    nc.vector.tensor_mul(ckvT[:, cs], pa[96:128, :], rb[96:128, :])

            # ========== phase C: projections + rope + assembly ==========
            with tc.tile_pool(name="psC", bufs=2, space="PSUM") as psC:

                def proj3(pn, pxr, pxi, wn, wxr, wxi, rhs_aps):
                    """rhs_aps: list of [K,512] APs (one accumulation step each)."""
                    for ps_, w_ in ((pn, wn), (pxr, wxr), (pxi, wxi)):
                        ws = w_ if isinstance(w_, list) else [w_]
                        for k, (wt, ra) in enumerate(zip(ws, rhs_aps)):
                            nc.tensor.matmul(ps_, wt[:, :], ra,
                                             start=(k == 0), stop=(k == len(rhs_aps) - 1))

                def rope_scatter(dsts, pn, m1, m2, m3_, m4, cs, width):
                    for h in range(H):
                        hs = ds(32 * h, 32)
                        nc.scalar.copy(dsts[h][0:32, cs], pn[32 * h:32 * h + 32, 0:width])
                        nc.gpsimd.tensor_sub(dsts[h][32:64, cs], m1[hs, 0:width], m2[hs, 0:width])
                        nc.gpsimd.tensor_add(dsts[h][64:96, cs], m3_[hs, 0:width], m4[hs, 0:width])

                def rope_muls(pxr, pxi, cos_t, sin_t, rs, width):
                    m1 = mt.tile([128, 512], BF, tag="m1")
                    m2 = mt.tile([128, 512], BF, tag="m2")
                    m3_ = mt.tile([128, 512], BF, tag="m3t")
                    m4 = mt.tile([128, 512], BF, tag="m4")
                    nc.vector.tensor_mul(m1[:, 0:width], pxr[:, 0:width], cos_t[:, rs])
                    nc.vector.tensor_mul(m2[:, 0:width], pxi[:, 0:width], sin_t[:, rs])
                    nc.vector.tensor_mul(m3_[:, 0:width], pxr[:, 0:width], sin_t[:, rs])
                    nc.vector.tensor_mul(m4[:, 0:width], pxi[:, 0:width], cos_t[:, rs])
                    return m1, m2, m3_, m4

                # Q (rhs = cq rows of cn, contraction 96)
                for jc in range(NJ):
                    rs = ds(512 * jc, 512)
                    pn = psC.tile([128, 512], F32, tag="pn")
                    pxr = psC.tile([128, 512], F32, tag="pxr")
                    pxi = psC.tile([128, 512], F32, tag="pxi")
                    proj3(pn[:, :], pxr[:, :], pxi[:, :], sb["wqn"], sb["wqxr"], sb["wqxi"],
                          [cqT[:, rs]])
                    ms = rope_muls(pxr, pxi, sb["cos4"], sb["sin4"], rs, 512)
                    rope_scatter(qT, pn, *ms, rs, 512)

                # KW (rhs = xT, two slices)
                for jc in range(NJ):
                    rs = ds(512 * jc, 512)
                    pn = psC.tile([128, 512], F32, tag="pn")
                    pxr = psC.tile([128, 512], F32, tag="pxr")
                    pxi = psC.tile([128, 512], F32, tag="pxi")
                    proj3(pn[:, :], pxr[:, :], pxi[:, :], sb["wwn"], sb["wwxr"], sb["wwxi"],
                          [xT0[:, rs], xT1[:, rs]])
                    ms = rope_muls(pxr, pxi, sb["cos4"], sb["sin4"], rs, 512)
                    rope_scatter(kwT, pn, *ms, rs, 512)

                # KS (rhs = selT, two slices, one 512 chunk)
                rs = ds(0, 512)
                pn = psC.tile([128, 512], F32, tag="pn")
                pxr = psC.tile([128, 512], F32, tag="pxr")
                pxi = psC.tile([128, 512], F32, tag="pxi")
                proj3(pn[:, :], pxr[:, :], pxi[:, :], sb["wsn"], sb["wsxr"], sb["wsxi"],
                      [selT0[:, rs], selT1[:, rs]])
                ms = rope_muls(pxr, pxi, sb["csel"], sb["ssel"], rs, 512)
                rope_scatter(ksT, pn, *ms, rs, 512)

                # K1: kn from ckv rows of cn; kr from xT (shared across heads)
                for jc in range(NJ):
                    cs = ds(512 * jc, 512)
                    pn = psC.tile([128, 512], F32, tag="pn")
                    nc.tensor.matmul(pn[:, :], sb["wkn"][:, :], ckvT[:, cs], start=True, stop=True)
                    pxr = psC.tile([128, 512], F32, tag="pxr")
                    pxi = psC.tile([128, 512], F32, tag="pxi")
                    nc.tensor.matmul(pxr[0:32, :], sb["wkrxr"][0][:, :], xT0[:, cs], start=True, stop=False)
                    nc.tensor.matmul(pxr[0:32, :], sb["wkrxr"][1][:, :], xT1[:, cs], start=False, stop=True)
                    nc.tensor.matmul(pxi[0:32, :], sb["wkrxi"][0][:, :], xT0[:, cs], start=True, stop=False)
                    nc.tensor.matmul(pxi[0:32, :], sb["wkrxi"][1][:, :], xT1[:, cs], start=False, stop=True)
                    t1 = mt.tile([32, 512], BF, tag="t1")
                    t2 = mt.tile([32, 512], BF, tag="t2")
                    t3 = mt.tile([32, 512], BF, tag="t3")
                    t4 = mt.tile([32, 512], BF, tag="t4")
                    nc.vector.tensor_mul(t1[:, :], pxr[0:32, :], sb["cos4"][0:32, cs])
                    nc.vector.tensor_mul(t2[:, :], pxi[0:32, :], sb["sin4"][0:32, cs])
                    nc.vector.tensor_mul(t3[:, :], pxr[0:32, :], sb["sin4"][0:32, cs])
                    nc.vector.tensor_mul(t4[:, :], pxi[0:32, :], sb["cos4"][0:32, cs])
                    krr = mt.tile([32, 512], BF, tag="krr")
                    kri = mt.tile([32, 512], BF, tag="kri")
                    nc.vector.tensor_sub(krr[:, :], t1[:, :], t2[:, :])
                    nc.vector.tensor_add(kri[:, :], t3[:, :], t4[:, :])
                    for h in range(H):
                        nc.scalar.copy(k1T[h][0:32, cs], pn[32 * h:32 * h + 32, :])
                        nc.gpsimd.tensor_copy(k1T[h][32:64, cs], krr[:, :])
                        nc.gpsimd.tensor_copy(k1T[h][64:96, cs], kri[:, :])

                # V tiles + [V | invg] assembly
                def v_assemble(dst, w_, rhs2, ntile, invg_col, from_cn=False):
                    for tt in range(ntile):
                        pv = psC.tile([128, 128], F32, tag="pvv")
                        if from_cn:
                            nc.tensor.matmul(pv[:, :], ckvT[:, ts(tt, 128)], w_[:, :], start=True, stop=True)
                        else:
                            nc.tensor.matmul(pv[:, :], rhs2[0][:, ts(tt, 128)], w_[0][:, :], start=True, stop=False)
                            nc.tensor.matmul(pv[:, :], rhs2[1][:, ts(tt, 128)], w_[1][:, :], start=False, stop=True)
                        for h in range(H):
                            nc.vector.tensor_copy(dst[h][:, ds(64 * tt, 32)], pv[:, ds(32 * h, 32)])
                            nc.gpsimd.tensor_copy(dst[h][:, ds(64 * tt + 32, 32)], sb["invg"][:, ds(invg_col, 32)])

                v_assemble(vg1, sb["wv"], None, 16, 0, from_cn=True)
                v_assemble(vg2, sb["wsv"], (selT0, selT1), 4, 32)
                v_assemble(vg3, sb["wwv"], (xT0, xT1), 16, 64)

            # ================= phase D: attention =================
            with (
                tc.tile_pool(name="psDs", bufs=3, space="PSUM") as psDs,
                tc.tile_pool(name="psDpv", bufs=3, space="PSUM") as psDpv,
            ):
                for h in range(H):
                    for jc in range(NJ):
                        tqs = ds(512 * jc, 512)
                        os_ = ocT[32 * h:32 * h + 32, tqs]
                        # ---- branch 1: causal full ----
                        n1 = 4 * jc + 4
                        pv1 = psDpv.tile([64, 512], F32, tag="pv")
                        for i in range(n1):
                            s = psDs.tile([128, 512], F32, tag="s")
                            nc.tensor.matmul(s[:, :], k1T[h][:, ts(i, 128)], qT[h][:, tqs], start=True, stop=True)
                            if i >= 4 * jc:
                                w = (i - 4 * jc + 1) * 128
                                nc.vector.tensor_add(s[:, 0:w], s[:, 0:w], sb["mb"][:, 512 - w:512])
                            p = wk.tile([128, 512], BF, tag="p")
                            nc.scalar.activation(p[:, :], s[:, :], AF.Exp, scale=SCALE)
                            nc.tensor.matmul(pv1[:, :], vg1[h][:, ts(i, 64)], p[:, :],
                                             start=(i == 0), stop=(i == n1 - 1))
                        r1 = wk.tile([32, 512], F32, tag="r")
                        _recip(r1[:, :], pv1[32:64, :])
                        nc.vector.tensor_mul(os_, pv1[0:32, :], r1[:, :])
                        # ---- branch 2: selected tokens ----
                        pv2 = psDpv.tile([64, 512], F32, tag="pv")
                        for i in range(4):
                            s = psDs.tile([128, 512], F32, tag="s")
                            nc.tensor.matmul(s[:, :], ksT[h][:, ts(i, 128)], qT[h][:, tqs], start=True, stop=True)
                            p = wk.tile([128, 512], BF, tag="p")
                            nc.scalar.activation(p[:, :], s[:, :], AF.Exp, scale=SCALE)
                            nc.tensor.matmul(pv2[:, :], vg2[h][:, ts(i, 64)], p[:, :],
                                             start=(i == 0), stop=(i == 3))
                        r2 = wk.tile([32, 512], F32, tag="r")
                        _recip(r2[:, :], pv2[32:64, :])
                        t2_ = wk.tile([128, 512], BF, tag="tt")
                        hs_ = slice(32 * h, 32 * h + 32)
                        nc.vector.tensor_mul(t2_[hs_, :], pv2[0:32, :], r2[:, :])
                        nc.vector.tensor_add(os_, os_, t2_[hs_, :])
                        # ---- branch 3: sliding window ----
                        for jj in range(4):
                            j = 4 * jc + jj
                            tqj = ds(128 * j, 128)
                            pv3 = psDpv.tile([64, 512], F32, tag="pv")
                            contrib = [i for i in (j - 1, j) if i >= 0]
                            for n_, i in enumerate(contrib):
                                s3 = psDs.tile([128, 512], F32, tag="s")
                                nc.tensor.matmul(s3[:, 0:128], kwT[h][:, ts(i, 128)], qT[h][:, tqj], start=True, stop=True)
                                off = 128 * (j - i)
                                nc.vector.tensor_add(s3[:, 0:128], s3[:, 0:128], sb["m3"][:, ds(off, 128)])
                                p3 = wk.tile([128, 128], BF, tag="p3")
                                nc.scalar.activation(p3[:, :], s3[:, 0:128], AF.Exp, scale=SCALE)
                                nc.tensor.matmul(pv3[:, 0:128], vg3[h][:, ts(i, 64)], p3[:, :],
                                                 start=(n_ == 0), stop=(n_ == len(contrib) - 1))
                            r3 = wk.tile([32, 128], F32, tag="r3")
                            _recip(r3[:, :], pv3[32:64, 0:128])
                            t3_ = wk.tile([128, 128], BF, tag="t3b")
                            nc.vector.tensor_mul(t3_[hs_, :], pv3[0:32, 0:128], r3[:, :])
                            nc.vector.tensor_add(ocT[32 * h:32 * h + 32, tqj],
                                                 ocT[32 * h:32 * h + 32, tqj], t3_[hs_, :])

            if debug_outs:
                for nm, tile_ in (("d_cqT", cqT), ("d_ckvT", ckvT), ("d_qT0", qT[0]),
                                  ("d_k1T0", k1T[0]), ("d_kwT0", kwT[0]), ("d_ksT0", ksT[0]),
                                  ("d_vg10", vg1[0]), ("d_ocT", ocT)):
                    nc.sync.dma_start(out=dbg[nm][:, :], in_=tile_[:, :])

            # ================= phase E: output projection =================
            with tc.tile_pool(name="psE", bufs=2, space="PSUM") as psE:
                for tt in range(T // 128):
                    po = psE.tile([128, 256], F32, tag="po")
                    nc.tensor.matmul(po[:, :], ocT[:, ts(tt, 128)], sb["wproj"][:, :], start=True, stop=True)
                    ob = wk.tile([128, 256], F32, tag="ob")
                    nc.scalar.copy(ob[:, :], po[:, :])
                    nc.sync.dma_start(out=out_dram[ts(tt, 128), :], in_=ob[:, :])

    nc.finalize()
    return nc


# ---------------------------------------------------------------------------
# runner
# ---------------------------------------------------------------------------

_CACHE = {}


def _host_prep(inputs):
    x = inputs["x"]
    cos, sin = _freqs()
    gate = _softmax((x @ inputs["Wgate"]).mean(1), -1)  # [B, 3]
    scores = np.einsum("btc,c->bt", x, inputs["W_imp"][:, 0])
    idx = np.sort(np.argpartition(-scores, KEEP - 1, axis=1)[:, :KEEP], axis=1)
    sel = np.take_along_axis(x, idx[..., None], 1)  # [B, KEEP, C]
    return dict(gate=gate, idx=idx, sel=sel, cos=cos, sin=sin)


FAST_RECIP = False


def kernel(**inputs):
    inputs = {k: np.asarray(v, dtype=np.float32) for k, v in inputs.items()}
    host = _host_prep(inputs)

    if "nc" not in _CACHE:
        _CACHE["nc"] = build_nc(fast_recip=FAST_RECIP)
    nc = _CACHE["nc"]

    in_maps = []
    for core in range(NCORES):
        b, hg = divmod(core, 4)
        in_maps.append(_prep_core_inputs(inputs, b, hg, host))

    from concourse.bass_utils import run_bass_kernel_spmd
    res = run_bass_kernel_spmd(nc, in_maps, core_ids=list(range(NCORES)))

    B = inputs["x"].shape[0]
    out = np.zeros((B, T, C), np.float32)
    for core in range(NCORES):
        b = core // 4
        out[b] += res.results[core]["out"]
    return out


# revision 20
# speedup vs baseline: 1.2419x; 1.2419x over previous
"""nn_Attn_9715216024104 — sparse attention (MLA + top-k select + sliding window)
on 8 Trainium2 NeuronCores via Bass/Tile.

Sharding: core = b*4 + hg  (b = batch 0..1, hg = head-group 0..3, 4 heads each).
Each core computes its 4 heads' three attention branches end-to-end in bf16
(fp32 PSUM accumulation) and its row-slice of Wproj; the host sums the four
partial [T, C] outputs per batch.

Device layout highlights:
  - activations kept transposed ([dim, T]) so scores come out as S^T[tk, tq];
    softmax row-sums are PE ones-columns appended to the PV lhsT ([V | 1/g]),
    which also folds the branch gate into the normalization.
  - no max-subtraction in softmax (scores are O(1)); masking is additive -600
    pre-scale constants (exp -> exact 0 past bf16).
  - rms norm via Square / ones-matmul / exp(-0.5 ln(.)) (single ACT table set),
    with the norm weights folded into Wcq/Wckv on the host.
  - rope computed as XR*cos -+ XI*sin with real/imag weight columns pre-split
    on the host so the multiplies run at full 128-partition width.

Host side (cheap, O(T*C)): top-k token selection + gather, gate softmax,
transposes, bf16 casts, weight slicing/folding. Device does all O(T^2) work.
"""

import math

import numpy as np
import ml_dtypes

N_HEAD = 16
NOPE, ROPE, VDIM = 32, 64, 32
HD = 96
WINDOW = 128
KEEP = 512
EPS = 1e-6
T, C = 2048, 256
H = 4  # heads per core
NCORES = 8
MASKVAL = -600.0
SCALE = 1.0 / math.sqrt(HD)

BF16 = ml_dtypes.bfloat16


# ---------------------------------------------------------------------------
# host-side helpers
# ---------------------------------------------------------------------------

def _freqs(t=T, dim=ROPE, theta=1e4):
    f = 1.0 / theta ** (np.arange(0, dim, 2, dtype=np.float32) / dim)
    ang = np.outer(np.arange(t, dtype=np.float32), f)
    return np.cos(ang).astype(np.float32), np.sin(ang).astype(np.float32)


def _softmax(s, axis=-1):
    m = np.max(s, axis=axis, keepdims=True)
    e = np.exp(s - m)
    return e / e.sum(axis=axis, keepdims=True)


def _consts():
    """Inline (NEFF-baked) constant tensors."""
    cos, sin = _freqs()
    cos4 = np.tile(cos.T, (H, 1))  # [128, T]
    sin4 = np.tile(sin.T, (H, 1))
    p = np.arange(128)
    mb = np.full((128, 512), MASKVAL, np.float32)
    mb[:, 384:] = np.where(p[:, None] <= p[None, :], 0.0, MASKVAL)
    cc = np.arange(256)
    m3 = np.where((cc[None, :] >= p[:, None]) & (cc[None, :] < p[:, None] + 128),
                  0.0, MASKVAL).astype(np.float32)
    sql = np.zeros((128, 2), np.float32)
    sql[:96, 0] = 1.0   # cq rows
    sql[96:, 1] = 1.0   # ckv rows
    rstdbc = np.zeros((2, 128), np.float32)
    rstdbc[0, :96] = 1.0
    rstdbc[1, 96:] = 1.0
    scl2 = np.array([[1.0 / 96.0, EPS], [1.0 / 32.0, EPS]], np.float32)
    bf = lambda a: a.astype(BF16)
    return dict(cos4=bf(cos4), sin4=bf(sin4), mb=bf(mb), m3=bf(m3),
                sql=bf(sql), rstdbc=bf(rstdbc), scl2=scl2)


def _prep_core_inputs(inputs, b, hg, host):
    """Build the per-core in_map (all bf16; out is f32)."""
    bf = lambda a: np.ascontiguousarray(a).astype(BF16)
    h0 = hg * H
    x = inputs["x"][b]
    m = {}
    m["xT"] = bf(x.T)
    m["selT"] = bf(host["sel"][b].T)
    m["csel"] = bf(np.tile(host["cos"][host["idx"][b]].T, (H, 1)))
    m["ssel"] = bf(np.tile(host["sin"][host["idx"][b]].T, (H, 1)))
    invg = (1.0 / host["gate"][b]).astype(np.float32)  # [3]
    for br in range(3):
        blk = np.zeros((128, 64), np.float32)
        blk[:, :32] = invg[br]
        m[f"vginit{br}"] = bf(np.tile(blk, (1, 16)))  # [128, 1024]

    wcq = inputs["Wcq"] * inputs["q_norm_w"][None, :]
    wckv = inputs["Wckv"] * inputs["kv_norm_w"][None, :]
    m["wA"] = bf(np.concatenate([wcq, wckv], 1))  # [256, 128]: cq cols 0-95, ckv 96-127

    wqn3 = inputs["Wq_nope"].reshape(96, N_HEAD, NOPE)[:, h0:h0 + H]
    wqr3 = inputs["Wq_rope"].reshape(96, N_HEAD, ROPE)[:, h0:h0 + H]
    m["wqn"] = bf(wqn3.reshape(96, H * NOPE))
    m["wqxr"] = bf(wqr3[..., :32].reshape(96, H * 32))
    m["wqxi"] = bf(wqr3[..., 32:].reshape(96, H * 32))

    m["wkn"] = bf(inputs["Wk_nope"].reshape(32, N_HEAD, NOPE)[:, h0:h0 + H].reshape(32, H * NOPE))
    m["wv"] = bf(inputs["Wv"].reshape(32, N_HEAD, VDIM)[:, h0:h0 + H].reshape(32, H * VDIM))
    wkr = inputs["Wk_rope"] / N_HEAD
    m["wkrxr"] = bf(wkr[:, :32])
    m["wkrxi"] = bf(wkr[:, 32:])

    for tag, wk_, wv_ in (("s", "Wsel_k", "Wsel_v"), ("w", "Wwin_k", "Wwin_v")):
        k3 = inputs[wk_].reshape(C, N_HEAD, HD)[:, h0:h0 + H]
        m[f"w{tag}n"] = bf(k3[..., :32].reshape(C, H * 32))
        m[f"w{tag}xr"] = bf(k3[..., 32:64].reshape(C, H * 32))
        m[f"w{tag}xi"] = bf(k3[..., 64:].reshape(C, H * 32))
        m[f"w{tag}v"] = bf(inputs[wv_].reshape(C, N_HEAD, VDIM)[:, h0:h0 + H].reshape(C, H * VDIM))

    m["wproj"] = bf(inputs["Wproj"][h0 * VDIM:(h0 + H) * VDIM])  # [128, 256]
    return m


# ---------------------------------------------------------------------------
# device program
# ---------------------------------------------------------------------------

def build_nc(debug_outs=False, fast_recip=True):
    import concourse.bacc as bacc
    import concourse.mybir as mybir
    from concourse.bass import ts, ds
    from concourse.tile import TileContext

    BF = mybir.dt.bfloat16
    F32 = mybir.dt.float32
    AF = mybir.ActivationFunctionType

    nc = bacc.Bacc("TRN2")

    def _recip(out, in_):
        if fast_recip:
            nc.vector.reciprocal_approx_fast(out=out, in_=in_)
        else:
            nc.vector.reciprocal(out=out, in_=in_)

    P = {}
    def inp(name, shape, dt=BF):
        P[name] = nc.declare_dram_parameter(name, list(shape), dt, isOutput=False)

    inp("xT", [C, T]); inp("selT", [C, KEEP])
    inp("csel", [128, KEEP]); inp("ssel", [128, KEEP])
    for br in range(3):
        inp(f"vginit{br}", [128, 1024])
    inp("wA", [C, 128])
    inp("wqn", [96, 128]); inp("wqxr", [96, 128]); inp("wqxi", [96, 128])
    inp("wkn", [32, 128]); inp("wv", [32, 128])
    inp("wkrxr", [C, 32]); inp("wkrxi", [C, 32])
    inp("wsn", [C, 128]); inp("wsxr", [C, 128]); inp("wsxi", [C, 128]); inp("wsv", [C, 128])
    inp("wwn", [C, 128]); inp("wwxr", [C, 128]); inp("wwxi", [C, 128]); inp("wwv", [C, 128])
    inp("wproj", [128, C])
    out_dram = nc.declare_dram_parameter("out", [T, C], F32, isOutput=True)
    dbg = {}
    if debug_outs:
        for nm, sh in (("d_cqT", [96, T]), ("d_ckvT", [32, T]), ("d_qT0", [96, T]),
                       ("d_k1T0", [96, T]), ("d_kwT0", [96, T]), ("d_ksT0", [96, KEEP]),
                       ("d_vg10", [128, 1024]), ("d_ocT", [128, T])):
            dbg[nm] = nc.declare_dram_parameter(nm, sh, BF, isOutput=True)

    cst = _consts()
    inl = {k: nc.inline_tensor(v, name=f"c_{k}") for k, v in cst.items()}

    NJ = T // 512  # 4 tq chunks of 512

    with TileContext(nc) as tc:
        with (
            tc.tile_pool(name="persist", bufs=1) as pp,
            tc.tile_pool(name="work", bufs=3) as wk,
            tc.tile_pool(name="mtiles", bufs=2) as mt,
        ):
            # ---- load params + consts to SBUF ----
            sb = {}
            for name, dram in P.items():
                if name.startswith("vginit"):
                    continue
                sh = list(dram.shape)
                if sh[0] > 128:  # split partition dim
                    tls = []
                    for i in range(sh[0] // 128):
                        t_ = pp.tile([128, sh[1]], BF, name=f"{name}_{i}")
                        nc.sync.dma_start(out=t_[:, :], in_=dram[ts(i, 128), :])
                        tls.append(t_)
                    sb[name] = tls
                else:
                    t_ = pp.tile(sh, BF, name=f"{name}_sb")
                    nc.sync.dma_start(out=t_[:, :], in_=dram[:, :])
                    sb[name] = t_
            for name in ("cos4", "sin4", "mb", "m3", "sql", "rstdbc"):
                d = inl[name]
                t_ = pp.tile(list(d.shape), BF, name=f"{name}_sb")
                nc.sync.dma_start(out=t_[:, :], in_=d[:, :])
                sb[name] = t_
            scl2_sb = pp.tile([2, 2], F32, name="scl2_sb")
            nc.sync.dma_start(out=scl2_sb[:, :], in_=inl["scl2"][:, :])

            xT0, xT1 = sb["xT"]
            selT0, selT1 = sb["selT"]

            # ---- persistent activation tiles ----
            cqT = pp.tile([96, T], BF, name="cqT")
            ckvT = pp.tile([32, T], BF, name="ckvT")
            qT = [pp.tile([96, T], BF, name=f"qT{h}") for h in range(H)]
            k1T = [pp.tile([96, T], BF, name=f"k1T{h}") for h in range(H)]
            kwT = [pp.tile([96, T], BF, name=f"kwT{h}") for h in range(H)]
            ksT = [pp.tile([96, KEEP], BF, name=f"ksT{h}") for h in range(H)]
            vg1 = [pp.tile([128, 16 * 64], BF, name=f"vg1_{h}") for h in range(H)]
            vg2 = [pp.tile([128, 4 * 64], BF, name=f"vg2_{h}") for h in range(H)]
            vg3 = [pp.tile([128, 16 * 64], BF, name=f"vg3_{h}") for h in range(H)]
            for h in range(H):
                nc.sync.dma_start(out=vg1[h][:, :], in_=P["vginit0"][:, :])
                nc.sync.dma_start(out=vg2[h][:, :], in_=P["vginit1"][:, 0:256])
                nc.sync.dma_start(out=vg3[h][:, :], in_=P["vginit2"][:, :])
            ocT = pp.tile([128, T], BF, name="ocT")

            # ================= phase B: cq/ckv + rms =================
            with tc.tile_pool(name="psB", bufs=2, space="PSUM") as psB:
                for jc in range(NJ):
                    cs = ds(512 * jc, 512)
                    pa = psB.tile([128, 512], F32, tag="pa")
                    nc.tensor.matmul(pa[:, :], sb["wA"][0][:, :], xT0[:, cs], start=True, stop=False)
                    nc.tensor.matmul(pa[:, :], sb["wA"][1][:, :], xT1[:, cs], start=False, stop=True)
                    sq = wk.tile([128, 512], BF, tag="sq")
                    nc.scalar.activation(sq[:, :], pa[:, :], AF.Square)
                    pss = psB.tile([2, 512], F32, tag="pss")
                    nc.tensor.matmul(pss[:, :], sb["sql"][:, :], sq[:, :], start=True, stop=True)
                    lnt = wk.tile([2, 512], F32, tag="lnt")
                    nc.scalar.activation(lnt[:, :], pss[:, :], AF.Ln, scale=scl2_sb[0:2, 0:1], bias=scl2_sb[0:2, 1:2])
                    rstd = wk.tile([2, 512], BF, tag="rstd")
                    nc.scalar.activation(rstd[:, :], lnt[:, :], AF.Exp, scale=-0.5)
                    pr = psB.tile([128, 512], F32, tag="pr")
                    nc.tensor.matmul(pr[:, :], sb["rstdbc"][:, :], rstd[:, :], start=True, stop=True)
                    rb = wk.tile([128, 512], BF, tag="rb")
                    nc.scalar.copy(rb[:, :], pr[:, :])
                    nc.vector.tensor_mul(cqT[:, cs], pa[0:96, :], rb[0:96, :])
                    nc.vector.tensor_mul(ckvT[:, cs], pa[96:128, :], rb[96:128, :])

            # ========== phase C: projections + rope + assembly ==========
            with tc.tile_pool(name="psC", bufs=2, space="PSUM") as psC:

                def proj3(pn, pxr, pxi, wn, wxr, wxi, rhs_aps):
                    """rhs_aps: list of [K,512] APs (one accumulation step each)."""
                    for ps_, w_ in ((pn, wn), (pxr, wxr), (pxi, wxi)):
                        ws = w_ if isinstance(w_, list) else [w_]
                        for k, (wt, ra) in enumerate(zip(ws, rhs_aps)):
                            nc.tensor.matmul(ps_, wt[:, :], ra,
                                             start=(k == 0), stop=(k == len(rhs_aps) - 1))

                def rope_scatter(dsts, pn, m1, m2, m3_, m4, cs, width):
                    for h in range(H):
                        hs = ds(32 * h, 32)
                        nc.scalar.copy(dsts[h][0:32, cs], pn[32 * h:32 * h + 32, 0:width])
                        nc.gpsimd.tensor_sub(dsts[h][32:64, cs], m1[hs, 0:width], m2[hs, 0:width])
                        nc.gpsimd.tensor_add(dsts[h][64:96, cs], m3_[hs, 0:width], m4[hs, 0:width])

                def rope_muls(pxr, pxi, cos_t, sin_t, rs, width):
                    m1 = mt.tile([128, 512], BF, tag="m1")
                    m2 = mt.tile([128, 512], BF, tag="m2")
                    m3_ = mt.tile([128, 512], BF, tag="m3t")
                    m4 = mt.tile([128, 512], BF, tag="m4")
                    nc.vector.tensor_mul(m1[:, 0:width], pxr[:, 0:width], cos_t[:, rs])
                    nc.vector.tensor_mul(m2[:, 0:width], pxi[:, 0:width], sin_t[:, rs])
                    nc.vector.tensor_mul(m3_[:, 0:width], pxr[:, 0:width], sin_t[:, rs])
                    nc.vector.tensor_mul(m4[:, 0:width], pxi[:, 0:width], cos_t[:, rs])
                    return m1, m2, m3_, m4

                # Q (rhs = cq rows of cn, contraction 96)
                for jc in range(NJ):
                    rs = ds(512 * jc, 512)
                    pn = psC.tile([128, 512], F32, tag="pn")
                    pxr = psC.tile([128, 512], F32, tag="pxr")
                    pxi = psC.tile([128, 512], F32, tag="pxi")
                    proj3(pn[:, :], pxr[:, :], pxi[:, :], sb["wqn"], sb["wqxr"], sb["wqxi"],
                          [cqT[:, rs]])
                    ms = rope_muls(pxr, pxi, sb["cos4"], sb["sin4"], rs, 512)
                    rope_scatter(qT, pn, *ms, rs, 512)

                # KW (rhs = xT, two slices)
                for jc in range(NJ):
                    rs = ds(512 * jc, 512)
                    pn = psC.tile([128, 512], F32, tag="pn")
                    pxr = psC.tile([128, 512], F32, tag="pxr")
                    pxi = psC.tile([128, 512], F32, tag="pxi")
                    proj3(pn[:, :], pxr[:, :], pxi[:, :], sb["wwn"], sb["wwxr"], sb["wwxi"],
                          [xT0[:, rs], xT1[:, rs]])
                    ms = rope_muls(pxr, pxi, sb["cos4"], sb["sin4"], rs, 512)
                    rope_scatter(kwT, pn, *ms, rs, 512)

                # KS (rhs = selT, two slices, one 512 chunk)
                rs = ds(0, 512)
                pn = psC.tile([128, 512], F32, tag="pn")
                pxr = psC.tile([128, 512], F32, tag="pxr")
                pxi = psC.tile([128, 512], F32, tag="pxi")
                proj3(pn[:, :], pxr[:, :], pxi[:, :], sb["wsn"], sb["wsxr"], sb["wsxi"],
                      [selT0[:, rs], selT1[:, rs]])
                ms = rope_muls(pxr, pxi, sb["csel"], sb["ssel"], rs, 512)
                rope_scatter(ksT, pn, *ms, rs, 512)

                # K1: kn from ckv rows of cn; kr from xT (shared across heads)
                for jc in range(NJ):
                    cs = ds(512 * jc, 512)
                    pn = psC.tile([128, 512], F32, tag="pn")
                    nc.tensor.matmul(pn[:, :], sb["wkn"][:, :], ckvT[:, cs], start=True, stop=True)
                    pxr = psC.tile([128, 512], F32, tag="pxr")
                    pxi = psC.tile([128, 512], F32, tag="pxi")
                    nc.tensor.matmul(pxr[0:32, :], sb["wkrxr"][0][:, :], xT0[:, cs], start=True, stop=False)
                    nc.tensor.matmul(pxr[0:32, :], sb["wkrxr"][1][:, :], xT1[:, cs], start=False, stop=True)
                    nc.tensor.matmul(pxi[0:32, :], sb["wkrxi"][0][:, :], xT0[:, cs], start=True, stop=False)
                    nc.tensor.matmul(pxi[0:32, :], sb["wkrxi"][1][:, :], xT1[:, cs], start=False, stop=True)
                    t1 = mt.tile([32, 512], BF, tag="t1")
                    t2 = mt.tile([32, 512], BF, tag="t2")
                    t3 = mt.tile([32, 512], BF, tag="t3")
                    t4 = mt.tile([32, 512], BF, tag="t4")
                    nc.vector.tensor_mul(t1[:, :], pxr[0:32, :], sb["cos4"][0:32, cs])
                    nc.vector.tensor_mul(t2[:, :], pxi[0:32, :], sb["sin4"][0:32, cs])
                    nc.vector.tensor_mul(t3[:, :], pxr[0:32, :], sb["sin4"][0:32, cs])
                    nc.vector.tensor_mul(t4[:, :], pxi[0:32, :], sb["cos4"][0:32, cs])
                    krr = mt.tile([32, 512], BF, tag="krr")
                    kri = mt.tile([32, 512], BF, tag="kri")
                    nc.vector.tensor_sub(krr[:, :], t1[:, :], t2[:, :])
                    nc.vector.tensor_add(kri[:, :], t3[:, :], t4[:, :])
                    for h in range(H):
                        nc.scalar.copy(k1T[h][0:32, cs], pn[32 * h:32 * h + 32, :])
                        nc.gpsimd.tensor_copy(k1T[h][32:64, cs], krr[:, :])
                        nc.gpsimd.tensor_copy(k1T[h][64:96, cs], kri[:, :])

                # V tiles + [V | invg] assembly
                def v_assemble(dst, w_, rhs2, ntile, invg_col, from_cn=False):
                    for tt in range(ntile):
                        pv = psC.tile([128, 128], F32, tag="pvv")
                        if from_cn:
                            nc.tensor.matmul(pv[:, :], ckvT[:, ts(tt, 128)], w_[:, :], start=True, stop=True)
                        else:
                            nc.tensor.matmul(pv[:, :], rhs2[0][:, ts(tt, 128)], w_[0][:, :], start=True, stop=False)
                            nc.tensor.matmul(pv[:, :], rhs2[1][:, ts(tt, 128)], w_[1][:, :], start=False, stop=True)
                        for h in range(H):
                            nc.vector.tensor_copy(dst[h][:, ds(64 * tt + 32, 32)], pv[:, ds(32 * h, 32)])

                v_assemble(vg1, sb["wv"], None, 16, 0, from_cn=True)  # [invg | V] blocks
                v_assemble(vg2, sb["wsv"], (selT0, selT1), 4, 32)
                v_assemble(vg3, sb["wwv"], (xT0, xT1), 16, 64)

            # ================= phase D: attention =================
            with (
                tc.tile_pool(name="psDs", bufs=3, space="PSUM") as psDs,
                tc.tile_pool(name="psDpv", bufs=3, space="PSUM") as psDpv,
            ):
                for h in range(H):
                    for jc in range(NJ):
                        tqs = ds(512 * jc, 512)
                        hs_ = slice(32 * h, 32 * h + 32)
                        os_ = ocT[hs_, tqs]
                        # ---- branch 1: causal full (left-of-diagonal columns skipped) ----
                        n1 = 4 * jc + 4
                        pv1 = psDpv.tile([64, 512], F32, tag="pv")
                        for i in range(n1):
                            r = i - 4 * jc
                            c0 = 128 * r if r > 0 else 0
                            wv_ = 512 - c0
                            s = psDs.tile([128, 512], F32, tag="s")
                            nc.tensor.matmul(s[:, c0:512], k1T[h][:, ts(i, 128)],
                                             qT[h][:, ds(512 * jc + c0, wv_)], start=True, stop=True)
                            if r >= 0:
                                nc.vector.tensor_add(s[:, c0:c0 + 128], s[:, c0:c0 + 128],
                                                     sb["m3"][:, 0:128])
                            p = wk.tile([128, 512], BF, tag="p")
                            nc.scalar.activation(p[:, c0:512], s[:, c0:512], AF.Exp, scale=SCALE)
                            nc.tensor.matmul(pv1[:, c0:512], vg1[h][:, ts(i, 64)], p[:, c0:512],
                                             start=(i == 0), stop=(i == n1 - 1))
                        r1 = wk.tile([32, 512], F32, tag="r")
                        _recip(r1[:, :], pv1[0:32, :])
                        nc.vector.tensor_mul(os_, pv1[32:64, :], r1[:, :])
                        # ---- branch 2: selected tokens ----
                        pv2 = psDpv.tile([64, 512], F32, tag="pv")
                        for i in range(4):
                            s = psDs.tile([128, 512], F32, tag="s")
                            nc.tensor.matmul(s[:, :], ksT[h][:, ts(i, 128)], qT[h][:, tqs], start=True, stop=True)
                            p = wk.tile([128, 512], BF, tag="p")
                            nc.scalar.activation(p[:, :], s[:, :], AF.Exp, scale=SCALE)
                            nc.tensor.matmul(pv2[:, :], vg2[h][:, ts(i, 64)], p[:, :],
                                             start=(i == 0), stop=(i == 3))
                        r2 = wk.tile([32, 512], F32, tag="r")
                        _recip(r2[:, :], pv2[0:32, :])
                        t2_ = wk.tile([128, 512], BF, tag="tt")
                        nc.vector.tensor_mul(t2_[hs_, :], pv2[32:64, :], r2[:, :])
                        nc.vector.tensor_add(os_, os_, t2_[hs_, :])
                        # ---- branch 3: sliding window ----
                        # one [128, 256] S tile per contributing tk tile (i = 4jc-1 .. 4jc+3);
                        # each 128-wide query tile j consumes halves of two P tiles.
                        i_lo = max(4 * jc - 1, 0)
                        p3s = {}
                        for i in range(i_lo, 4 * jc + 4):
                            w3 = min(256, T - 128 * i)
                            s3 = psDs.tile([128, 512], F32, tag="s")
                            nc.tensor.matmul(s3[:, 0:w3], kwT[h][:, ts(i, 128)],
                                             qT[h][:, ds(128 * i, w3)], start=True, stop=True)
                            nc.vector.tensor_add(s3[:, 0:w3], s3[:, 0:w3], sb["m3"][:, 0:w3])
                            p3 = wk.tile([128, 256], BF, tag="p3", bufs=7)
                            nc.scalar.activation(p3[:, 0:w3], s3[:, 0:w3], AF.Exp, scale=SCALE)
                            p3s[i] = p3
                        pv3 = psDpv.tile([64, 512], F32, tag="pv")
                        for jj in range(4):
                            j = 4 * jc + jj
                            contrib = [i for i in (j - 1, j) if i >= 0]
                            for n_, i in enumerate(contrib):
                                off = 128 * (j - i)  # 0 (own tile) or 128 (right half of prev)
                                nc.tensor.matmul(pv3[:, ds(128 * jj, 128)], vg3[h][:, ts(i, 64)],
                                                 p3s[i][:, ds(off, 128)],
                                                 start=(n_ == 0), stop=(n_ == len(contrib) - 1))
                        r3 = wk.tile([32, 512], F32, tag="r")
                        _recip(r3[:, :], pv3[0:32, :])
                        t3_ = wk.tile([128, 512], BF, tag="tt")
                        nc.vector.tensor_mul(t3_[hs_, :], pv3[32:64, :], r3[:, :])
                        nc.vector.tensor_add(os_, os_, t3_[hs_, :])

            if debug_outs:
                for nm, tile_ in (("d_cqT", cqT), ("d_ckvT", ckvT), ("d_qT0", qT[0]),
                                  ("d_k1T0", k1T[0]), ("d_kwT0", kwT[0]), ("d_ksT0", ksT[0]),
                                  ("d_vg10", vg1[0]), ("d_ocT", ocT)):
                    nc.sync.dma_start(out=dbg[nm][:, :], in_=tile_[:, :])

            # ================= phase E: output projection =================
            with tc.tile_pool(name="psE", bufs=2, space="PSUM") as psE:
                for tt in range(T // 128):
                    po = psE.tile([128, 256], F32, tag="po")
                    nc.tensor.matmul(po[:, :], ocT[:, ts(tt, 128)], sb["wproj"][:, :], start=True, stop=True)
                    ob = wk.tile([128, 256], F32, tag="ob")
                    nc.scalar.copy(ob[:, :], po[:, :])
                    nc.sync.dma_start(out=out_dram[ts(tt, 128), :], in_=ob[:, :])

    nc.finalize()
    return nc


# ---------------------------------------------------------------------------
# runner
# ---------------------------------------------------------------------------

_CACHE = {}


def _host_prep(inputs):
    x = inputs["x"]
    cos, sin = _freqs()
    gate = _softmax((x @ inputs["Wgate"]).mean(1), -1)  # [B, 3]
    scores = np.einsum("btc,c->bt", x, inputs["W_imp"][:, 0])
    idx = np.sort(np.argpartition(-scores, KEEP - 1, axis=1)[:, :KEEP], axis=1)
    sel = np.take_along_axis(x, idx[..., None], 1)  # [B, KEEP, C]
    return dict(gate=gate, idx=idx, sel=sel, cos=cos, sin=sin)


FAST_RECIP = True


def kernel(**inputs):
    inputs = {k: np.asarray(v, dtype=np.float32) for k, v in inputs.items()}
    host = _host_prep(inputs)

    if "nc" not in _CACHE:
        _CACHE["nc"] = build_nc(fast_recip=FAST_RECIP)
    nc = _CACHE["nc"]

    in_maps = []
    for core in range(NCORES):
        b, hg = divmod(core, 4)
        in_maps.append(_prep_core_inputs(inputs, b, hg, host))

    from concourse.bass_utils import run_bass_kernel_spmd
    res = run_bass_kernel_spmd(nc, in_maps, core_ids=list(range(NCORES)))

    B = inputs["x"].shape[0]
    out = np.zeros((B, T, C), np.float32)
    for core in range(NCORES):
        b = core // 4
        out[b] += res.results[core]["out"]
    return out


# revision 27
# speedup vs baseline: 1.4732x; 1.1862x over previous
"""nn_Attn_9715216024104 — sparse attention (MLA + top-k select + sliding window)
on 8 Trainium2 NeuronCores via Bass/Tile.

Sharding: core = b*4 + hg  (b = batch 0..1, hg = head-group 0..3, 4 heads each).
Each core computes its 4 heads' three attention branches end-to-end in bf16
(fp32 PSUM accumulation) and its row-slice of Wproj; the host sums the four
partial [T, C] outputs per batch.

Device layout highlights:
  - activations kept transposed ([dim, T]) so scores come out as S^T[tk, tq];
    softmax row-sums are PE ones-columns appended to the PV lhsT ([V | 1/g]),
    which also folds the branch gate into the normalization.
  - no max-subtraction in softmax (scores are O(1)); masking is additive -600
    pre-scale constants (exp -> exact 0 past bf16).
  - rms norm via Square / ones-matmul / exp(-0.5 ln(.)) (single ACT table set),
    with the norm weights folded into Wcq/Wckv on the host.
  - rope computed as XR*cos -+ XI*sin with real/imag weight columns pre-split
    on the host so the multiplies run at full 128-partition width.

Host side (cheap, O(T*C)): top-k token selection + gather, gate softmax,
transposes, bf16 casts, weight slicing/folding. Device does all O(T^2) work.
"""

import math

import numpy as np
import ml_dtypes

N_HEAD = 16
NOPE, ROPE, VDIM = 32, 64, 32
HD = 96
WINDOW = 128
KEEP = 512
EPS = 1e-6
T, C = 2048, 256
H = 4  # heads per core
NCORES = 8
MASKVAL = -600.0
SCALE = 1.0 / math.sqrt(HD)

BF16 = ml_dtypes.bfloat16


# ---------------------------------------------------------------------------
# host-side helpers
# ---------------------------------------------------------------------------

def _freqs(t=T, dim=ROPE, theta=1e4):
    f = 1.0 / theta ** (np.arange(0, dim, 2, dtype=np.float32) / dim)
    ang = np.outer(np.arange(t, dtype=np.float32), f)
    return np.cos(ang).astype(np.float32), np.sin(ang).astype(np.float32)


def _softmax(s, axis=-1):
    m = np.max(s, axis=axis, keepdims=True)
    e = np.exp(s - m)
    return e / e.sum(axis=axis, keepdims=True)


def _register_exp64():
    """Register the custom DVE exp op: out = (in0*s0 + in1)^64.
    With s0 = SCALE/64 and in1 = 1 (valid) / 0 (masked) this computes
    exp(score*SCALE) to ~1% for |score*SCALE| <~ 1, and exactly 0 for
    masked positions."""
    from concourse.dve_spec import Spec, Src0, Src1, C0, sq
    from concourse import dve_ops
    for op in dve_ops.OPS:
        if op.name == "EXP64_ANT":
            return op
    body = sq(sq(sq(sq(sq(sq(Src0 * C0 + Src1))))))
    def ref(in0, in1, c0, c1, c2):
        t = in0.astype(np.float32) * np.float32(c0) + in1.astype(np.float32)
        for _ in range(6):
            t = t * t
        return t
    op = dve_ops.DveOp("EXP64_ANT", Spec(body=body, reference=ref), subdim=False,
                       uops_sha={"v3": "0ff874fad7093ae6"})
    dve_ops.OPS.append(op)
    dve_ops.CUSTOM_DVE_SPECS["EXP64_ANT"] = op.spec
    dve_ops._SUB_OPCODE_FOR_NAME["EXP64_ANT"] = (
        dve_ops._CUSTOM_DVE_ROW_BASE + len(dve_ops.OPS) - 1)
    return op


def _consts():
    """Inline (NEFF-baked) constant tensors."""
    cos, sin = _freqs()
    cos4 = np.tile(cos.T, (H, 1))  # [128, T]
    sin4 = np.tile(sin.T, (H, 1))
    p = np.arange(128)
    mb = np.full((128, 512), MASKVAL, np.float32)
    mb[:, 384:] = np.where(p[:, None] <= p[None, :], 0.0, MASKVAL)
    cc = np.arange(256)
    m3 = np.where((cc[None, :] >= p[:, None]) & (cc[None, :] < p[:, None] + 128),
                  0.0, MASKVAL).astype(np.float32)
    sql = np.zeros((128, 2), np.float32)
    sql[:96, 0] = 1.0   # cq rows
    sql[96:, 1] = 1.0   # ckv rows
    rstdbc = np.zeros((2, 128), np.float32)
    rstdbc[0, :96] = 1.0
    rstdbc[1, 96:] = 1.0
    scl2 = np.array([[1.0 / 96.0, EPS], [1.0 / 32.0, EPS]], np.float32)
    m3e = np.where((cc[None, :] >= p[:, None]) & (cc[None, :] < p[:, None] + 128),
                   1.0, 0.0).astype(np.float32)
    m1e = np.ones((128, 1024), np.float32)
    m1e[:, 0:128] = (p[:, None] <= np.arange(128)[None, :]).astype(np.float32)
    onec = np.ones((128, 1), np.float32)
    bf = lambda a: a.astype(BF16)
    return dict(cos4=bf(cos4), sin4=bf(sin4), mb=bf(mb), m3=bf(m3),
                m3e=bf(m3e), m1e=bf(m1e), onec=bf(onec),
                sql=bf(sql), rstdbc=bf(rstdbc), scl2=scl2)


def _prep_core_inputs(inputs, b, hg, host):
    """Build the per-core in_map (all bf16; out is f32)."""
    bf = lambda a: np.ascontiguousarray(a).astype(BF16)
    h0 = hg * H
    x = inputs["x"][b]
    m = {}
    m["xT"] = bf(x.T)
    m["selT"] = bf(host["sel"][b].T)
    m["csel"] = bf(np.tile(host["cos"][host["idx"][b]].T, (H, 1)))
    m["ssel"] = bf(np.tile(host["sin"][host["idx"][b]].T, (H, 1)))
    invg = (1.0 / host["gate"][b]).astype(np.float32)  # [3]
    for br in range(3):
        blk = np.zeros((128, 64), np.float32)
        blk[:, :32] = invg[br]
        m[f"vginit{br}"] = bf(np.tile(blk, (1, 16)))  # [128, 1024]

    wcq = inputs["Wcq"] * inputs["q_norm_w"][None, :]
    wckv = inputs["Wckv"] * inputs["kv_norm_w"][None, :]
    m["wA"] = bf(np.concatenate([wcq, wckv], 1))  # [256, 128]: cq cols 0-95, ckv 96-127

    wqn3 = inputs["Wq_nope"].reshape(96, N_HEAD, NOPE)[:, h0:h0 + H]
    wqr3 = inputs["Wq_rope"].reshape(96, N_HEAD, ROPE)[:, h0:h0 + H]
    m["wqn"] = bf(wqn3.reshape(96, H * NOPE))
    m["wqxr"] = bf(wqr3[..., :32].reshape(96, H * 32))
    m["wqxi"] = bf(wqr3[..., 32:].reshape(96, H * 32))

    m["wkn"] = bf(inputs["Wk_nope"].reshape(32, N_HEAD, NOPE)[:, h0:h0 + H].reshape(32, H * NOPE))
    m["wv"] = bf(inputs["Wv"].reshape(32, N_HEAD, VDIM)[:, h0:h0 + H].reshape(32, H * VDIM))
    wkr = inputs["Wk_rope"] / N_HEAD
    m["wkrxr"] = bf(wkr[:, :32])
    m["wkrxi"] = bf(wkr[:, 32:])

    for tag, wk_, wv_ in (("s", "Wsel_k", "Wsel_v"), ("w", "Wwin_k", "Wwin_v")):
        k3 = inputs[wk_].reshape(C, N_HEAD, HD)[:, h0:h0 + H]
        m[f"w{tag}n"] = bf(k3[..., :32].reshape(C, H * 32))
        m[f"w{tag}xr"] = bf(k3[..., 32:64].reshape(C, H * 32))
        m[f"w{tag}xi"] = bf(k3[..., 64:].reshape(C, H * 32))
        m[f"w{tag}v"] = bf(inputs[wv_].reshape(C, N_HEAD, VDIM)[:, h0:h0 + H].reshape(C, H * VDIM))

    m["wproj"] = bf(inputs["Wproj"][h0 * VDIM:(h0 + H) * VDIM])  # [128, 256]
    return m


# ---------------------------------------------------------------------------
# device program
# ---------------------------------------------------------------------------

def build_nc(debug_outs=False, fast_recip=True, exp_route=None):
    import concourse.bacc as bacc
    import concourse.mybir as mybir
    from concourse.bass import ts, ds
    from concourse.tile import TileContext

    BF = mybir.dt.bfloat16
    F32 = mybir.dt.float32
    AF = mybir.ActivationFunctionType

    nc = bacc.Bacc("TRN2")

    exp_route = exp_route or EXP_ROUTE
    exp64 = _register_exp64()
    EXPC = float(SCALE / 64.0)

    def _recip(out, in_):
        if fast_recip:
            nc.vector.reciprocal_approx_fast(out=out, in_=in_)
        else:
            nc.vector.reciprocal(out=out, in_=in_)

    P = {}
    def inp(name, shape, dt=BF):
        P[name] = nc.declare_dram_parameter(name, list(shape), dt, isOutput=False)

    inp("xT", [C, T]); inp("selT", [C, KEEP])
    inp("csel", [128, KEEP]); inp("ssel", [128, KEEP])
    for br in range(3):
        inp(f"vginit{br}", [128, 1024])
    inp("wA", [C, 128])
    inp("wqn", [96, 128]); inp("wqxr", [96, 128]); inp("wqxi", [96, 128])
    inp("wkn", [32, 128]); inp("wv", [32, 128])
    inp("wkrxr", [C, 32]); inp("wkrxi", [C, 32])
    inp("wsn", [C, 128]); inp("wsxr", [C, 128]); inp("wsxi", [C, 128]); inp("wsv", [C, 128])
    inp("wwn", [C, 128]); inp("wwxr", [C, 128]); inp("wwxi", [C, 128]); inp("wwv", [C, 128])
    inp("wproj", [128, C])
    out_dram = nc.declare_dram_parameter("out", [T, C], F32, isOutput=True)
    dbg = {}
    if debug_outs:
        for nm, sh in (("d_cqT", [96, T]), ("d_ckvT", [32, T]), ("d_qT0", [96, T]),
                       ("d_k1T0", [96, T]), ("d_kwT0", [96, T]), ("d_ksT0", [96, KEEP]),
                       ("d_vg10", [128, 1024]), ("d_ocT", [128, T])):
            dbg[nm] = nc.declare_dram_parameter(nm, sh, BF, isOutput=True)

    cst = _consts()
    inl = {k: nc.inline_tensor(v, name=f"c_{k}") for k, v in cst.items()}

    NJ = T // 512  # 4 tq chunks of 512

    with TileContext(nc) as tc:
        with (
            tc.tile_pool(name="persist", bufs=1) as pp,
            tc.tile_pool(name="work", bufs=3) as wk,
            tc.tile_pool(name="mtiles", bufs=2) as mt,
        ):
            # ---- load params + consts to SBUF ----
            sb = {}
            for name, dram in P.items():
                if name.startswith("vginit"):
                    continue
                sh = list(dram.shape)
                if sh[0] > 128:  # split partition dim
                    tls = []
                    for i in range(sh[0] // 128):
                        t_ = pp.tile([128, sh[1]], BF, name=f"{name}_{i}")
                        nc.sync.dma_start(out=t_[:, :], in_=dram[ts(i, 128), :])
                        tls.append(t_)
                    sb[name] = tls
                else:
                    t_ = pp.tile(sh, BF, name=f"{name}_sb")
                    nc.sync.dma_start(out=t_[:, :], in_=dram[:, :])
                    sb[name] = t_
            for name in ("cos4", "sin4", "mb", "m3", "m3e", "m1e", "onec", "sql", "rstdbc"):
                d = inl[name]
                t_ = pp.tile(list(d.shape), BF, name=f"{name}_sb")
                nc.sync.dma_start(out=t_[:, :], in_=d[:, :])
                sb[name] = t_
            scl2_sb = pp.tile([2, 2], F32, name="scl2_sb")
            nc.sync.dma_start(out=scl2_sb[:, :], in_=inl["scl2"][:, :])

            xT0, xT1 = sb["xT"]
            selT0, selT1 = sb["selT"]

            # ---- persistent activation tiles ----
            cqT = pp.tile([96, T], BF, name="cqT")
            ckvT = pp.tile([32, T], BF, name="ckvT")
            qT = [pp.tile([96, T], BF, name=f"qT{h}") for h in range(H)]
            k1T = [pp.tile([96, T], BF, name=f"k1T{h}") for h in range(H)]
            kwT = [pp.tile([96, T], BF, name=f"kwT{h}") for h in range(H)]
            ksT = [pp.tile([96, KEEP], BF, name=f"ksT{h}") for h in range(H)]
            vg1 = [pp.tile([128, 16 * 64], BF, name=f"vg1_{h}") for h in range(H)]
            vg2 = [pp.tile([128, 4 * 64], BF, name=f"vg2_{h}") for h in range(H)]
            vg3 = [pp.tile([128, 16 * 64], BF, name=f"vg3_{h}") for h in range(H)]
            for h in range(H):
                nc.sync.dma_start(out=vg1[h][:, :], in_=P["vginit0"][:, :])
                nc.sync.dma_start(out=vg2[h][:, :], in_=P["vginit1"][:, 0:256])
                nc.sync.dma_start(out=vg3[h][:, :], in_=P["vginit2"][:, :])
            ocT = pp.tile([128, T], BF, name="ocT")

            # ================= phase B: cq/ckv + rms =================
            with tc.tile_pool(name="psB", bufs=2, space="PSUM") as psB:
                for jc in range(NJ):
                    cs = ds(512 * jc, 512)
                    pa = psB.tile([128, 512], F32, tag="pa")
                    nc.tensor.matmul(pa[:, :], sb["wA"][0][:, :], xT0[:, cs], start=True, stop=False)
                    nc.tensor.matmul(pa[:, :], sb["wA"][1][:, :], xT1[:, cs], start=False, stop=True)
                    sq = wk.tile([128, 512], BF, tag="sq")
                    nc.scalar.activation(sq[:, :], pa[:, :], AF.Square)
                    pss = psB.tile([2, 512], F32, tag="pss")
                    nc.tensor.matmul(pss[:, :], sb["sql"][:, :], sq[:, :], start=True, stop=True)
                    lnt = wk.tile([2, 512], F32, tag="lnt")
                    nc.scalar.activation(lnt[:, :], pss[:, :], AF.Ln, scale=scl2_sb[0:2, 0:1], bias=scl2_sb[0:2, 1:2])
                    rstd = wk.tile([2, 512], BF, tag="rstd")
                    nc.scalar.activation(rstd[:, :], lnt[:, :], AF.Exp, scale=-0.5)
                    pr = psB.tile([128, 512], F32, tag="pr")
                    nc.tensor.matmul(pr[:, :], sb["rstdbc"][:, :], rstd[:, :], start=True, stop=True)
                    rb = wk.tile([128, 512], BF, tag="rb")
                    nc.scalar.copy(rb[:, :], pr[:, :])
                    nc.vector.tensor_mul(cqT[:, cs], pa[0:96, :], rb[0:96, :])
                    nc.vector.tensor_mul(ckvT[:, cs], pa[96:128, :], rb[96:128, :])

            # ========== phase C: projections + rope + assembly ==========
            # rope products are accumulated into full-T sbuf tiles so the
            # per-head scatters run as a few big [32, T] ops instead of many
            # small per-chunk ones (gpsimd per-instruction overhead ~0.6us).
            MQ = [pp.tile([128, T], BF, name=f"MQ{k}") for k in range(4)]
            MW = [pp.tile([128, T], BF, name=f"MW{k}") for k in range(4)]
            with tc.tile_pool(name="psC", bufs=2, space="PSUM") as psC:

                def proj3(pn, pxr, pxi, wn, wxr, wxi, rhs_aps):
                    for ps_, w_ in ((pn, wn), (pxr, wxr), (pxi, wxi)):
                        ws = w_ if isinstance(w_, list) else [w_]
                        for k, (wt, ra) in enumerate(zip(ws, rhs_aps)):
                            nc.tensor.matmul(ps_, wt[:, :], ra,
                                             start=(k == 0), stop=(k == len(rhs_aps) - 1))

                def rope_muls(M, pxr, pxi, cos_t, sin_t, rs, cs, width, pd=128):
                    nc.vector.tensor_mul(M[0][0:pd, cs], pxr[:, 0:width], cos_t[:, rs])
                    nc.vector.tensor_mul(M[1][0:pd, cs], pxi[:, 0:width], sin_t[:, rs])
                    nc.vector.tensor_mul(M[2][0:pd, cs], pxr[:, 0:width], sin_t[:, rs])
                    nc.vector.tensor_mul(M[3][0:pd, cs], pxi[:, 0:width], cos_t[:, rs])

                def rope_scatter(dsts, M, cols):
                    for h in range(H):
                        hsl = ds(32 * h, 32)
                        nc.gpsimd.tensor_sub(dsts[h][32:64, 0:cols], M[0][hsl, 0:cols], M[1][hsl, 0:cols])
                        nc.gpsimd.tensor_add(dsts[h][64:96, 0:cols], M[2][hsl, 0:cols], M[3][hsl, 0:cols])

                # Q (rhs = cqT, contraction 96) and KW (rhs = xT, contraction 256)
                for M, dsts, wn, wxr, wxi, rhs_of in (
                    (MQ, qT, "wqn", "wqxr", "wqxi", "q"),
                    (MW, kwT, "wwn", "wwxr", "wwxi", "w"),
                ):
                    for jc in range(NJ):
                        rs = ds(512 * jc, 512)
                        pn = psC.tile([128, 512], F32, tag="pn")
                        pxr = psC.tile([128, 512], F32, tag="pxr")
                        pxi = psC.tile([128, 512], F32, tag="pxi")
                        rhs = [cqT[:, rs]] if rhs_of == "q" else [xT0[:, rs], xT1[:, rs]]
                        proj3(pn[:, :], pxr[:, :], pxi[:, :], sb[wn], sb[wxr], sb[wxi], rhs)
                        rope_muls(M, pxr, pxi, sb["cos4"], sb["sin4"], rs, rs, 512)
                        for h in range(H):
                            nc.scalar.copy(dsts[h][0:32, rs], pn[32 * h:32 * h + 32, 0:512])
                    rope_scatter(dsts, M, T)

                # KS (rhs = selT, one 512 chunk) — reuses MQ cols 0:512
                rs = ds(0, 512)
                pn = psC.tile([128, 512], F32, tag="pn")
                pxr = psC.tile([128, 512], F32, tag="pxr")
                pxi = psC.tile([128, 512], F32, tag="pxi")
                proj3(pn[:, :], pxr[:, :], pxi[:, :], sb["wsn"], sb["wsxr"], sb["wsxi"],
                      [selT0[:, rs], selT1[:, rs]])
                rope_muls(MQ, pxr, pxi, sb["csel"], sb["ssel"], rs, rs, 512)
                for h in range(H):
                    nc.scalar.copy(ksT[h][0:32, rs], pn[32 * h:32 * h + 32, 0:512])
                rope_scatter(ksT, MQ, KEEP)

                # K1: kn from ckvT; kr from xT (shared across heads) — kr rope
                # products land in MQ rows 0:32, combined at full T.
                for jc in range(NJ):
                    cs = ds(512 * jc, 512)
                    pn = psC.tile([128, 512], F32, tag="pn")
                    nc.tensor.matmul(pn[:, :], sb["wkn"][:, :], ckvT[:, cs], start=True, stop=True)
                    pxr = psC.tile([128, 512], F32, tag="pxr")
                    pxi = psC.tile([128, 512], F32, tag="pxi")
                    nc.tensor.matmul(pxr[0:32, :], sb["wkrxr"][0][:, :], xT0[:, cs], start=True, stop=False)
                    nc.tensor.matmul(pxr[0:32, :], sb["wkrxr"][1][:, :], xT1[:, cs], start=False, stop=True)
                    nc.tensor.matmul(pxi[0:32, :], sb["wkrxi"][0][:, :], xT0[:, cs], start=True, stop=False)
                    nc.tensor.matmul(pxi[0:32, :], sb["wkrxi"][1][:, :], xT1[:, cs], start=False, stop=True)
                    rope_muls(MW, pxr[0:32], pxi[0:32], sb["cos4"][0:32], sb["sin4"][0:32], cs, cs, 512, pd=32)
                    for h in range(H):
                        nc.scalar.copy(k1T[h][0:32, cs], pn[32 * h:32 * h + 32, :])
                krr = pp.tile([32, T], BF, name="krr")
                kri = pp.tile([32, T], BF, name="kri")
                nc.vector.tensor_sub(krr[:, :], MW[0][0:32, :], MW[1][0:32, :])
                nc.vector.tensor_add(kri[:, :], MW[2][0:32, :], MW[3][0:32, :])
                for h in range(H):
                    nc.gpsimd.tensor_copy(k1T[h][32:64, :], krr[:, :])
                    nc.gpsimd.tensor_copy(k1T[h][64:96, :], kri[:, :])

                # V tiles + [invg | V] assembly (invg columns DMA-initialized)
                def v_assemble(dst, w_, rhs2, ntile, from_cn=False):
                    for tt in range(ntile):
                        pv = psC.tile([128, 128], F32, tag="pvv")
                        if from_cn:
                            nc.tensor.matmul(pv[:, :], ckvT[:, ts(tt, 128)], w_[:, :], start=True, stop=True)
                        else:
                            nc.tensor.matmul(pv[:, :], rhs2[0][:, ts(tt, 128)], w_[0][:, :], start=True, stop=False)
                            nc.tensor.matmul(pv[:, :], rhs2[1][:, ts(tt, 128)], w_[1][:, :], start=False, stop=True)
                        for h in range(H):
                            nc.vector.tensor_copy(dst[h][:, ds(64 * tt + 32, 32)], pv[:, ds(32 * h, 32)])

                v_assemble(vg1, sb["wv"], None, 16, from_cn=True)
                v_assemble(vg2, sb["wsv"], (selT0, selT1), 4)
                v_assemble(vg3, sb["wwv"], (xT0, xT1), 16)

            # ================= phase D: attention =================
            # tq processed in 1024-wide chunks (NJ2=2); PSUM s/pv tiles span 2 banks.
            NJ2 = T // 1024
            with (
                tc.tile_pool(name="psDs", bufs=2, space="PSUM") as psDs,
                tc.tile_pool(name="psDpv", bufs=2, space="PSUM") as psDpv,
            ):
                for h in range(H):
                    for jc in range(NJ2):
                        tqs = ds(1024 * jc, 1024)
                        hs_ = slice(32 * h, 32 * h + 32)
                        os_ = ocT[hs_, tqs]
                        # ---- branch 1: causal full (left-of-diagonal columns skipped) ----
                        n1 = 8 * jc + 8
                        pv1 = psDpv.tile([64, 1024], F32, tag="pv")
                        for i in range(n1):
                            r = i - 8 * jc
                            c0 = 128 * r if r > 0 else 0
                            wv_ = 1024 - c0
                            s = psDs.tile([128, 1024], F32, tag="s")
                            for sg0, sg1 in ((c0, 512), (max(c0, 512), 1024)):
                                if sg0 >= sg1:
                                    continue
                                nc.tensor.matmul(s[:, sg0:sg1], k1T[h][:, ts(i, 128)],
                                                 qT[h][:, ds(1024 * jc + sg0, sg1 - sg0)],
                                                 start=True, stop=True)
                            p = wk.tile([128, 1024], BF, tag="p")
                            route = exp_route["b1d"] if r >= 0 else exp_route["b1f"]
                            if route == "act":
                                if r >= 0:
                                    nc.vector.tensor_add(s[:, c0:c0 + 128], s[:, c0:c0 + 128],
                                                         sb["m3"][:, 0:128])
                                nc.scalar.activation(p[:, c0:1024], s[:, c0:1024], AF.Exp, scale=SCALE)
                            else:
                                in1 = sb["m1e"][:, 0:1024 - c0] if r >= 0 else sb["onec"][:, 0:1]
                                nc.vector._custom_dve(exp64, out=p[:, c0:1024], in0=s[:, c0:1024],
                                                      in1=in1, s0=EXPC)
                            # stop goes on each PSUM bank's final writer
                            # (bank 0 cols 0:512 -> diag r==3; bank 1 -> r==7)
                            for sg0, sg1, last_r in ((c0, 512, 3), (max(c0, 512), 1024, 7)):
                                if sg0 >= sg1:
                                    continue
                                nc.tensor.matmul(pv1[:, sg0:sg1], vg1[h][:, ts(i, 64)], p[:, sg0:sg1],
                                                 start=(i == 0), stop=(r == last_r))
                        r1 = wk.tile([32, 1024], F32, tag="r")
                        _recip(r1[:, :], pv1[0:32, :])
                        nc.vector.tensor_mul(os_, pv1[32:64, :], r1[:, :])
                        # ---- branch 2: selected tokens ----
                        pv2 = psDpv.tile([64, 1024], F32, tag="pv")
                        for i in range(4):
                            s = psDs.tile([128, 1024], F32, tag="s")
                            for sg in (0, 512):
                                nc.tensor.matmul(s[:, sg:sg + 512], ksT[h][:, ts(i, 128)],
                                                 qT[h][:, ds(1024 * jc + sg, 512)], start=True, stop=True)
                            p = wk.tile([128, 1024], BF, tag="p")
                            if exp_route["b2"] == "act":
                                nc.scalar.activation(p[:, :], s[:, :], AF.Exp, scale=SCALE)
                            else:
                                nc.vector._custom_dve(exp64, out=p[:, :], in0=s[:, :],
                                                      in1=sb["onec"][:, 0:1], s0=EXPC)
                            for sg in (0, 512):
                                nc.tensor.matmul(pv2[:, sg:sg + 512], vg2[h][:, ts(i, 64)], p[:, sg:sg + 512],
                                                 start=(i == 0), stop=(i == 3))
                        r2 = wk.tile([32, 1024], F32, tag="r")
                        _recip(r2[:, :], pv2[0:32, :])
                        t2_ = wk.tile([128, 1024], BF, tag="tt")
                        nc.vector.tensor_mul(t2_[hs_, :], pv2[32:64, :], r2[:, :])
                        nc.vector.tensor_add(os_, os_, t2_[hs_, :])
                        # ---- branch 3: sliding window ----
                        # one [128, 256] S tile per contributing tk tile; each 128-wide
                        # query tile j consumes halves of two P tiles.
                        i_lo = max(8 * jc - 1, 0)
                        p3s = {}
                        for i in range(i_lo, 8 * jc + 8):
                            w3 = min(256, T - 128 * i)
                            s3 = psDs.tile([128, 1024], F32, tag="s")
                            nc.tensor.matmul(s3[:, 0:w3], kwT[h][:, ts(i, 128)],
                                             qT[h][:, ds(128 * i, w3)], start=True, stop=True)
                            p3 = wk.tile([128, 256], BF, tag="p3", bufs=11)
                            if exp_route["b3"] == "act":
                                nc.vector.tensor_add(s3[:, 0:w3], s3[:, 0:w3], sb["m3"][:, 0:w3])
                                nc.scalar.activation(p3[:, 0:w3], s3[:, 0:w3], AF.Exp, scale=SCALE)
                            else:
                                nc.vector._custom_dve(exp64, out=p3[:, 0:w3], in0=s3[:, 0:w3],
                                                      in1=sb["m3e"][:, 0:w3], s0=EXPC)
                            p3s[i] = p3
                        pv3 = psDpv.tile([64, 1024], F32, tag="pv")
                        for jj in range(8):
                            j = 8 * jc + jj
                            contrib = [i for i in (j - 1, j) if i >= 0]
                            for n_, i in enumerate(contrib):
                                off = 128 * (j - i)
                                nc.tensor.matmul(pv3[:, ds(128 * jj, 128)], vg3[h][:, ts(i, 64)],
                                                 p3s[i][:, ds(off, 128)],
                                                 start=(n_ == 0), stop=(n_ == len(contrib) - 1))
                        r3 = wk.tile([32, 1024], F32, tag="r")
                        _recip(r3[:, :], pv3[0:32, :])
                        t3_ = wk.tile([128, 1024], BF, tag="tt")
                        nc.vector.tensor_mul(t3_[hs_, :], pv3[32:64, :], r3[:, :])
                        nc.vector.tensor_add(os_, os_, t3_[hs_, :])

            if debug_outs:
                for nm, tile_ in (("d_cqT", cqT), ("d_ckvT", ckvT), ("d_qT0", qT[0]),
                                  ("d_k1T0", k1T[0]), ("d_kwT0", kwT[0]), ("d_ksT0", ksT[0]),
                                  ("d_vg10", vg1[0]), ("d_ocT", ocT)):
                    nc.sync.dma_start(out=dbg[nm][:, :], in_=tile_[:, :])

            # ================= phase E: output projection =================
            with tc.tile_pool(name="psE", bufs=2, space="PSUM") as psE:
                for tt in range(T // 128):
                    po = psE.tile([128, 256], F32, tag="po")
                    nc.tensor.matmul(po[:, :], ocT[:, ts(tt, 128)], sb["wproj"][:, :], start=True, stop=True)
                    ob = wk.tile([128, 256], F32, tag="ob")
                    nc.scalar.copy(ob[:, :], po[:, :])
                    nc.sync.dma_start(out=out_dram[ts(tt, 128), :], in_=ob[:, :])

    nc.finalize()
    return nc


# ---------------------------------------------------------------------------
# runner
# ---------------------------------------------------------------------------

_CACHE = {}


def _host_prep(inputs):
    x = inputs["x"]
    cos, sin = _freqs()
    gate = _softmax((x @ inputs["Wgate"]).mean(1), -1)  # [B, 3]
    scores = np.einsum("btc,c->bt", x, inputs["W_imp"][:, 0])
    idx = np.sort(np.argpartition(-scores, KEEP - 1, axis=1)[:, :KEEP], axis=1)
    sel = np.take_along_axis(x, idx[..., None], 1)  # [B, KEEP, C]
    return dict(gate=gate, idx=idx, sel=sel, cos=cos, sin=sin)


FAST_RECIP = True
EXP_ROUTE = {"b1d": "act", "b1f": "act", "b2": "dve", "b3": "dve"}


def kernel(**inputs):
    inputs = {k: np.asarray(v, dtype=np.float32) for k, v in inputs.items()}
    host = _host_prep(inputs)

    if "nc" not in _CACHE:
        _CACHE["nc"] = build_nc(fast_recip=FAST_RECIP, exp_route=EXP_ROUTE)
    nc = _CACHE["nc"]

    in_maps = []
    for core in range(NCORES):
        b, hg = divmod(core, 4)
        in_maps.append(_prep_core_inputs(inputs, b, hg, host))

    from concourse.bass_utils import run_bass_kernel_spmd
    res = run_bass_kernel_spmd(nc, in_maps, core_ids=list(range(NCORES)))

    B = inputs["x"].shape[0]
    out = np.zeros((B, T, C), np.float32)
    for core in range(NCORES):
        b = core // 4
        out[b] += res.results[core]["out"]
    return out
